# revision 32
# baseline (speedup 1.0000x reference)
"""Trainium2 Bass kernel for GCL contrastive-scoring GNN message passing.

Computation (see the reference):
  h   = x @ W + b                      [N, H]
  q   = sigmoid(h)                     [N, H]
  k_p = normalize(segsum(pw*h))        [Np, H]
  k_n = segsum(pw*q)                   [Np, H]
  att = exp(k_p @ k_p.T / T)           [Np, Np]
  pos = (att * A_P) @ k_n ; neg = att @ k_n
  loss = mean(-log(q.pos[par]) + log(q.neg[par]))

v3: fp8(e4m3) DoubleRow matmuls throughout; paired [128,1024] PSUM
tiles for sigmoid and messages; segsum(pw*x) computed directly in
transposed [d, par] layout (no per-window transpose chain); messages
staged to a DRAM table and per-node rows fetched with SWDGE dma_gather
(Pool engine), so scores are all-SBUF fused multiply-accumulates on DVE
with no PSUM extraction and no one-hot gather matmuls.

Scale conventions (all cancel in the final loss):
  W staged as 8W      (sigmoid applies 1/8; k_p normalization cancels it)
  k_p staged as 16*k_p (exp scale absorbs 1/256)
  msgs staged as msg/512 (log-difference cancels the constant)
"""

import numpy as np
import ml_dtypes

import concourse.bass as bass
import concourse.bacc as bacc
import concourse.mybir as mybir
import concourse.tile as tile
from concourse import bass_utils

F32 = mybir.dt.float32
BF16 = mybir.dt.bfloat16
F8 = mybir.dt.float8e4
I16 = mybir.dt.int16
AF = mybir.ActivationFunctionType
ALU = mybir.AluOpType
DR = mybir.MatmulPerfMode.DoubleRow

NCORES = 8
NP = 4096          # parents
BAND = 512         # parents per core
NW = 4             # windows (128 parents) per core
D = 512
H = 512
KT = 4             # 128-tiles along D/H
NJ = NP // 128     # 32 j tiles
XW = (2 * KT + 1) * 128   # per-tile xs row: x d-major | seg 1-hot | x node-major
XCH = 4            # xs tiles per DMA chunk (must be even)
EPS = 1e-12
SCL_H = 8.0        # h staged as 8h
SCL_KP = 16.0      # kp staged as 16 kp
SCL_MSG = 1.0 / 512.0
EXP_SCALE = 2.0 / (SCL_KP * SCL_KP)   # exp((16kp_i . 16kp_j) * s) = exp(2 cos)
NF8 = ml_dtypes.float8_e4m3


# ----------------------------------------------------------------- host prep

def prep_inputs(x, node_to_par, p_weight, A_P, W, b):
    x = np.asarray(x, np.float32)
    par = np.asarray(node_to_par).astype(np.int64)
    pw = np.asarray(p_weight, np.float32)
    A_P = np.asarray(A_P, np.float32)
    W = np.asarray(W, np.float32)
    b = np.asarray(b, np.float32)
    N = x.shape[0]

    # group nodes by (core, window); per-window quota uniform, even # tiles
    grp = par // 128                       # [N] in [0, 32)
    order = np.argsort(grp, kind="stable")
    grp_sorted = grp[order]
    bounds = np.searchsorted(grp_sorted, np.arange(NCORES * NW + 1))
    counts = np.diff(bounds)
    QT = int(np.ceil(max(1, counts.max()) / 128))
    QT += QT % 2                           # even, for DoubleRow tile pairs
    Q0 = QT * 128
    T = NW * QT
    n_pad = NW * Q0

    bias_nonzero = bool(np.any(b != 0.0))

    W8 = np.clip(W * SCL_H, -240, 240).astype(NF8)  # [D, H]
    per_core = []
    for c in range(NCORES):
        slots = np.full(n_pad, -1, np.int64)
        for w in range(NW):
            g = c * NW + w
            lo, hi = bounds[g], bounds[g + 1]
            slots[w * Q0: w * Q0 + (hi - lo)] = order[lo:hi]
        valid = slots >= 0
        sc = np.where(valid, slots, 0)

        # xs[t, p, 0:4, n]   = x[node(t,n), k*128+p]   (d-major, encoder lhsT)
        # xs[t, p, 4, n]     = seg one-hot (pw)        (node on partitions)
        # xs[t, p, 5:9, :]   = x[node(t,p), :]         (node-major, segsum lhsT)
        x_sel = np.where(valid[:, None], x[sc], 0.0).astype(np.float32)
        xs = np.empty((T, 128, 2 * KT + 1, 128), NF8)
        xs[:, :, :KT, :] = x_sel.reshape(T, 128, KT, 128).transpose(0, 3, 2, 1)
        xs[:, :, KT + 1:, :] = x_sel.reshape(T, 128, KT, 128)

        wslot = np.arange(n_pad) // Q0
        cols = np.where(valid, par[sc] - c * BAND - wslot * 128, 0)
        rows = np.arange(n_pad)

        seg = np.zeros((n_pad, 128), np.float32)
        seg[rows[valid], cols[valid]] = pw[sc[valid]]
        xs[:, :, KT, :] = seg.reshape(T, 128, 128).astype(NF8)

        # per-slot band-local parent row index for the msgs-table gather
        # (int16, wrapped in 16 partitions, replicated to 128)
        bloc = np.where(valid, par[sc] - c * BAND, 0).astype(np.int16)
        idx16 = np.tile(
            np.ascontiguousarray(bloc.reshape(n_pad // 16, 16).T), (8, 1))

        maskT = np.ascontiguousarray(valid.reshape(T, 128).T).astype(np.float32)

        # A_P transposed band, laid out [p, jt*512+i]  (p = j within tile)
        A_PT = A_P[c * BAND:(c + 1) * BAND, :].T            # [4096 j, 512 i]
        aptA = np.ascontiguousarray(
            A_PT.reshape(NJ, 128, BAND).transpose(1, 0, 2).reshape(128, NJ * BAND)
        ).astype(NF8)

        per_core.append({
            "xs": np.ascontiguousarray(
                xs.reshape(T, 128, XW).transpose(1, 0, 2)
            ).reshape(128, T * XW),
            "idx16": idx16, "maskT": maskT, "aptA": aptA,
            "W8": np.ascontiguousarray(W8.reshape(KT, 128, H)),
            "ident": np.eye(128).astype(ml_dtypes.bfloat16),
            **({"bvec": (b * SCL_H).reshape(1, H).astype(ml_dtypes.bfloat16)}
               if bias_nonzero else {}),
        })
    meta = {"N": N, "T": T, "QT": QT, "n_pad": n_pad,
            "bias_nonzero": bias_nonzero}
    return per_core, meta


# ------------------------------------------------------------ device program

def build_program(T, QT, bias_nonzero, stage=4, reps=1, no_coll=False):
    nc = bacc.Bacc("TRN2", target_bir_lowering=False, debug=False,
                   num_devices=NCORES)

    n_pad = NW * QT * 128
    i_xs = nc.dram_tensor("xs", [128, T * XW], F8, kind="ExternalInput")
    i_idx = nc.dram_tensor("idx16", [128, n_pad // 16], I16,
                           kind="ExternalInput")
    i_mask = nc.dram_tensor("maskT", [128, T], F32, kind="ExternalInput")
    i_apt = nc.dram_tensor("aptA", [128, NJ * BAND], F8, kind="ExternalInput")
    i_w = nc.dram_tensor("W8", [KT, 128, H], F8, kind="ExternalInput")
    i_id = nc.dram_tensor("ident", [128, 128], BF16, kind="ExternalInput")
    if bias_nonzero:
        i_b = nc.dram_tensor("bvec", [1, H], BF16, kind="ExternalInput")
    o_loss = nc.dram_tensor("loss_part", [128, 1], F32, kind="ExternalOutput")

    # gather chunk tiles: one dma_gather writes CHT*128 SWDGE descriptors
    # into the 1024-entry ring (16KB scratch / 16B); stay safely under it
    CHT = 5

    with tile.TileContext(nc) as tc:
        with (
            tc.tile_pool(name="const", bufs=1) as constp,
            tc.tile_pool(name="bands", bufs=1) as bandp,
            tc.tile_pool(name="strm", bufs=3) as strm,
            tc.tile_pool(name="gsbp", bufs=2) as gsbp,
            tc.tile_pool(name="strm2", bufs=2) as strm2,
            tc.tile_pool(name="dram", bufs=1, space="DRAM") as dram,
        ):
            # constants / resident tensors
            w_sb = constp.tile([128, KT * H], F8, tag="w")
            for k in range(KT):
                nc.sync.dma_start(w_sb[:, k * H:(k + 1) * H], i_w[k])
            ident = constp.tile([128, 128], BF16, tag="ident")
            nc.sync.dma_start(ident[:], i_id[:])
            idx_sb = constp.tile([128, n_pad // 16], I16, tag="idx")
            nc.sync.dma_start(idx_sb[:], i_idx[:])
            apt_sb = constp.tile([128, NJ * BAND], F8, tag="apt")
            nc.sync.dma_start(apt_sb[:], i_apt[:])
            if bias_nonzero:
                bias_sb = constp.tile([1, H], BF16, tag="bias")
                nc.sync.dma_start(bias_sb[:], i_b[:])
                ones1 = constp.tile([1, 128], BF16, tag="ones1")
                nc.vector.memset(ones1[:], 1.0)
                onecol = constp.tile([128, 2], F8, tag="onecol")
                nc.vector.memset(onecol[:], 1.0)

            # persistent SBUF
            qres2 = [bandp.tile([128, T * H], F8, tag=f"qres{p}",
                                name=f"qres{p}")
                     for p in range(2)]
            kj_all = bandp.tile([128, NJ * 1024], F8, tag="kj_all")
            kpT2 = [bandp.tile([128, KT * BAND], F8, tag=f"kpT{p}",
                               name=f"kpT{p}")
                    for p in range(2)]
            kn82 = [bandp.tile([128, NW * H], F8, tag=f"kn8{p}",
                               name=f"kn8{p}")
                    for p in range(2)]
            scp2 = [bandp.tile([128, T], F32, tag=f"scp{p}", name=f"scp{p}")
                    for p in range(2)]
            scn2 = [bandp.tile([128, T], F32, tag=f"scn{p}", name=f"scn{p}")
                    for p in range(2)]

            agin = dram.tile([NW, 128, 1024], F8)

            w_r = w_sb[:].rearrange("p (k h) -> p k h", h=H)
            kj_r = kj_all[:].rearrange("p (j x) -> p j x", x=1024)

            for _rep in range(reps):
              qres = qres2[_rep % 2]
              kpT_band = kpT2[_rep % 2]
              kn8_band = kn82[_rep % 2]
              sc_pos, sc_neg = scp2[_rep % 2], scn2[_rep % 2]
              kpT_r = kpT_band[:].rearrange("p (s i) -> p s i", i=BAND)
              # per-rep Shared AllGather outputs (Shared DRAM wants exactly
              # one writing instruction per tensor)
              ag_kw = {} if no_coll else {"addr_space": "Shared"}
              agout = [dram.tile([NCORES, 128, 1024], F8,
                                 name=f"agout_{_rep}_{w}", **ag_kw)
                       for w in range(NW)]
              msgs_dram = dram.tile([BAND, 1024], F8, name=f"msgs_{_rep}")
              # ---------------- phase 1: encoder + segment sums ---------------
              ps1 = tc.tile_pool(name=f"ps1_{_rep}", bufs=1, space="PSUM")
              psA = ps1.__enter__()
              ssq4 = strm2.tile([128, NW], F32, tag="ssq4", bufs=1,
                                name=f"ssq4_{_rep}")
              kp_raw = [bandp.tile([128, H], BF16, tag=f"kpr{w}",
                                   name=f"kpr_{_rep}_{w}")
                        for w in range(NW)]
              for w in range(NW):
                  ps_kpxT = psA.tile([128, D], F32, tag="kpxT", bufs=1)
                  ps_kn = psA.tile([128, H], F32, tag="kn", bufs=1)
                  if bias_nonzero:
                      ps_spw = psA.tile([128, 1], F32, tag="spw", bufs=1)
                  for ti in range(QT):
                      t = w * QT + ti
                      if ti % XCH == 0:
                          nxc = min(XCH, QT - ti)
                          xch = strm.tile([128, XCH * XW], F8, tag="xch",
                                          bufs=3)
                          nc.gpsimd.dma_start(
                              xch[:, :nxc * XW],
                              i_xs[:, t * XW:(t + nxc) * XW])
                          xch_r = xch[:].rearrange("p (c xw) -> p c xw", xw=XW)
                      ci = ti % XCH

                      if ti % 2 == 0:
                          ps_h2 = psA.tile([128, 2 * H], F32, tag="h2",
                                           bufs=2)
                      hsl = ps_h2[:, (ti % 2) * H:(ti % 2 + 1) * H]
                      if bias_nonzero:
                          nc.tensor.matmul(hsl, ones1[:], bias_sb[:],
                                           start=True, stop=False)
                      xt_r = xch[:, ci * XW: ci * XW + KT * 128].rearrange(
                          "p (kk n) -> p kk n", n=128)
                      for kk in range(2):
                          nc.tensor.matmul(
                              hsl, xt_r[:, 2 * kk:2 * kk + 2, :],
                              w_r[:, 2 * kk:2 * kk + 2, :],
                              start=(kk == 0 and not bias_nonzero),
                              stop=(kk == 1), perf_mode=DR)

                      if ti % 2 == 1:
                          # paired sigmoid: the only per-pair PSUM read
                          nc.scalar.activation(
                              qres[:, (t - 1) * H:(t + 1) * H], ps_h2[:],
                              AF.Sigmoid, scale=1.0 / SCL_H)

                          oh_pair = xch_r[:, ci - 1:ci + 1,
                                          KT * 128:KT * 128 + 128]
                          xn_pair = xch_r[:, ci - 1:ci + 1,
                                          (KT + 1) * 128:XW]
                          q_pair = qres[:, (t - 1) * H:(t + 1) * H].rearrange(
                              "p (two h) -> p two h", two=2)
                          # transposed segsum(pw*x): out [d-chunk, par].
                          # one accumulation group for the whole bank: the
                          # first start lazy-zeroes the full 2KB zone
                          for cch in range(KT):
                              nc.tensor.matmul(
                                  ps_kpxT[:, cch * 128:(cch + 1) * 128],
                                  xn_pair[:, :, cch * 128:(cch + 1) * 128],
                                  oh_pair,
                                  start=(ti == 1 and cch == 0),
                                  stop=(ti == QT - 1 and cch == KT - 1),
                                  perf_mode=DR)
                          nc.tensor.matmul(ps_kn[:], oh_pair, q_pair,
                                           start=(ti == 1),
                                           stop=(ti == QT - 1), perf_mode=DR)
                          if bias_nonzero:
                              nc.tensor.matmul(ps_spw[:], oh_pair,
                                               onecol[:].rearrange(
                                                   "p (two c) -> p two c",
                                                   two=2),
                                               start=(ti == 1),
                                               stop=(ti == QT - 1),
                                               perf_mode=DR)

                  # kp = (segsum(pw*x))^T @ W  (+ segsum(pw) * b)
                  kpxT8 = strm2.tile([128, D], F8, tag="kpxT8")
                  nc.vector.tensor_copy(kpxT8[:], ps_kpxT[:])
                  ps_kp = psA.tile([128, H], F32, tag="kp", bufs=1)
                  kpxT8_r = kpxT8[:].rearrange("p (s q) -> p s q", q=128)
                  for kk in range(2):
                      nc.tensor.matmul(ps_kp[:], kpxT8_r[:, 2 * kk:2 * kk + 2],
                                       w_r[:, 2 * kk:2 * kk + 2, :],
                                       start=(kk == 0),
                                       stop=(kk == 1 and not bias_nonzero),
                                       perf_mode=DR)
                  if bias_nonzero:
                      spw8 = strm2.tile([128, 1], BF16, tag="spw8")
                      nc.scalar.copy(spw8[:], ps_spw[:])
                      ps_st = psA.tile([1, 128], BF16, tag="str", bufs=1)
                      nc.tensor.transpose(ps_st[:], spw8[:], ident[:])
                      spwT = strm2.tile([1, 128], BF16, tag="spwT")
                      nc.vector.tensor_copy(spwT[:], ps_st[:])
                      nc.tensor.matmul(ps_kp[:], spwT[:], bias_sb[:],
                                       start=False, stop=True)

                  # ssq via ACT Square+accum (same table set as Sigmoid);
                  # raw kp parked in SBUF; sqrt batched per window PAIR so
                  # the act table only switches twice per pair and the
                  # AllGather of windows {0,1} still overlaps windows {2,3}
                  tmp = strm2.tile([128, H], BF16, tag="nrm_tmp")
                  nc.scalar.activation(tmp[:], ps_kp[:], AF.Square,
                                       accum_out=ssq4[:, w:w + 1])
                  nc.vector.tensor_copy(kp_raw[w][:], ps_kp[:])
                  nc.vector.tensor_copy(kn8_band[:, w * H:(w + 1) * H],
                                        ps_kn[:])

                  if w % 2 == 0:
                      continue
                  # ---- normalize + transposes + AllGather for {w-1, w} ----
                  nrm2 = strm2.tile([128, 2], F32, tag="nrm2", bufs=2)
                  nc.scalar.activation(nrm2[:], ssq4[:, w - 1:w + 1], AF.Sqrt)
                  nc.vector.tensor_scalar_max(nrm2[:], nrm2[:], EPS)
                  rinv2 = strm2.tile([128, 2], F32, tag="rinv2", bufs=2)
                  nc.vector.reciprocal(rinv2[:], nrm2[:])
                  rinv16 = strm2.tile([128, 2], F32, tag="rinv16", bufs=2)
                  nc.vector.tensor_scalar_mul(rinv16[:], rinv2[:], SCL_KP)
                  for w2 in (w - 1, w):
                      kp_bf = strm2.tile([128, H], BF16, tag="kp_bf", bufs=2)
                      nc.vector.tensor_scalar_mul(
                          kp_bf[:], kp_raw[w2][:],
                          rinv16[:, w2 - w + 1:w2 - w + 2])
                      for s in range(KT):
                          ps_t = psA.tile([128, 128], BF16, tag="tr", bufs=1)
                          nc.tensor.transpose(
                              ps_t[:], kp_bf[:, s * 128:(s + 1) * 128],
                              ident[:])
                          nc.vector.tensor_copy(
                              kpT_band[:, s * BAND + w2 * 128:
                                       s * BAND + (w2 + 1) * 128],
                              ps_t[:])

                      if stage >= 2:
                          # stage agin[w2] = [kpT(s,q) | kn] and AllGather
                          nc.sync.dma_start(
                              agin[w2, :, 0:KT * 128].rearrange(
                                  "p (s q) -> p s q", q=128),
                              kpT_r[:, :, w2 * 128:(w2 + 1) * 128])
                          nc.sync.dma_start(agin[w2, :, KT * 128:1024],
                                            kn8_band[:, w2 * H:(w2 + 1) * H])
                          if no_coll:
                              for bb in range(NCORES):
                                  nc.sync.dma_start(agout[w2][bb], agin[w2])
                          else:
                              nc.gpsimd.collective_compute(
                                  "AllGather", ALU.bypass,
                                  replica_groups=[list(range(NCORES))],
                                  ins=[agin[w2].opt()],
                                  outs=[agout[w2][:].opt()])
              ps1.__exit__(None, None, None)

              if stage >= 3:
                  # -------- phase 2: attention + messages (+ phase 3) --------
                  ps2 = tc.tile_pool(name=f"ps2_{_rep}", bufs=2, space="PSUM")
                  psB = ps2.__enter__()

                  def phase2(ihalf):
                      acc2 = [psB.tile([128, 1024], F32, tag=f"acc{g}",
                                       bufs=1, name=f"acc{_rep}{ihalf}{g}")
                              for g in range(2)]
                      if ihalf == 0:
                          # one strided DMA per window: all 8 cores' slices
                          kj_v = kj_all[:].rearrange(
                              "p (b w x) -> p b w x", w=NW, x=1024)
                          for w2 in range(NW):
                              nc.sync.dma_start(
                                  kj_v[:, :, w2, :],
                                  agout[w2][:].rearrange("b p x -> p b x"))
                      for wh in range(2):
                          for bsel in range(NCORES):
                              jg0 = bsel * NW + 2 * wh
                              ps_att = psB.tile([128, 512], F32, tag="att")
                              # single accumulation group over both w2
                              # column halves (one 2KB zone, one start)
                              for w2 in range(2):
                                  jg = jg0 + w2
                                  for sp in range(2):
                                      lhsT = kj_all[
                                          :, jg * 1024 + sp * 256:
                                          jg * 1024 + (sp + 1) * 256].rearrange(
                                              "p (two q) -> p two q", two=2)
                                      nc.tensor.matmul(
                                          ps_att[:, w2 * 256:(w2 + 1) * 256],
                                          lhsT,
                                          kpT_r[:, 2 * sp:2 * sp + 2,
                                                ihalf * 256:(ihalf + 1) * 256],
                                          start=(w2 == 0 and sp == 0),
                                          stop=(w2 == 1 and sp == 1),
                                          perf_mode=DR)
                              attp = strm.tile([128, 512], F8, tag="attp",
                                               bufs=2)
                              nc.scalar.activation(attp[:], ps_att[:], AF.Exp,
                                                   scale=EXP_SCALE)
                              wposp = strm.tile([128, 512], F8, tag="wposp",
                                                bufs=2)
                              attp_r = attp[:].rearrange(
                                  "p (two i) -> p two i", two=2)
                              wposp_r = wposp[:].rearrange(
                                  "p (two i) -> p two i", two=2)
                              apt_pair = apt_sb[:].rearrange(
                                  "p (j i) -> p j i", i=BAND)[
                                  :, jg0:jg0 + 2,
                                  ihalf * 256:(ihalf + 1) * 256]
                              nc.vector.tensor_mul(wposp_r, attp_r, apt_pair)
                              kn_pair = kj_r[:, jg0:jg0 + 2, KT * 128:1024]
                              first = (wh == 0 and bsel == 0)
                              last = (wh == 1 and bsel == NCORES - 1)
                              for i2 in range(2):
                                  nc.tensor.matmul(
                                      acc2[i2][:, 0:512],
                                      wposp_r[:, :, i2 * 128:(i2 + 1) * 128],
                                      kn_pair, start=first, stop=last,
                                      perf_mode=DR)
                                  nc.tensor.matmul(
                                      acc2[i2][:, 512:1024],
                                      attp_r[:, :, i2 * 128:(i2 + 1) * 128],
                                      kn_pair, start=first, stop=last,
                                      perf_mode=DR)
                      for i2 in range(2):
                          g = ihalf * 2 + i2   # global i-sub == window index
                          msgs_sb = strm2.tile([128, 1024], F8,
                                               tag="msgs_sb", bufs=2)
                          nc.scalar.mul(msgs_sb[:], acc2[i2][:], SCL_MSG)
                          nc.sync.dma_start(
                              msgs_dram[g * 128:(g + 1) * 128, :], msgs_sb[:])

                  def phase3(ihalf):
                      # NOTE: keep ALL select math on DVE. A Pool
                      # tensor_mul + ACT accum-reduce offload balances the
                      # engines in the cost model (-35us) but measures 2x
                      # SLOWER on hardware: Pool alternates between the
                      # gather (mlp) and tensor ops (standard) gpsimd
                      # libraries, and the reloads are unmodeled and huge.
                      for w in (2 * ihalf, 2 * ihalf + 1):
                          ch0 = 0
                          while ch0 < QT:
                              nt = min(CHT, QT - ch0)
                              g_sb = gsbp.tile([128, CHT * 1024], F8,
                                               tag="gsb", bufs=2)
                              icol0 = (w * QT + ch0) * 8
                              nc.gpsimd.dma_gather(
                                  g_sb[:, :nt * 1024].rearrange(
                                      "p (c e) -> p c e", e=1024),
                                  msgs_dram[:, :],
                                  idx_sb[:, icol0:icol0 + nt * 8],
                                  nt * 128, nt * 128, 1024)
                              for tt in range(nt):
                                  t = w * QT + ch0 + tt
                                  qt = qres[:, t * H:(t + 1) * H]
                                  for sgn, sc_t in (
                                          (0, sc_pos), (1, sc_neg)):
                                      snk = strm.tile(
                                          [128, H], BF16,
                                          tag="snk", bufs=2)
                                      nc.vector.scalar_tensor_tensor(
                                          snk[:],
                                          g_sb[:, tt * 1024 + sgn * 512:
                                               tt * 1024 + (sgn + 1) * 512],
                                          1.0, qt,
                                          ALU.mult, ALU.mult,
                                          accum_out=sc_t[:, t:t + 1])
                              ch0 += nt

                  # emit both attention halves before the select tail so the
                  # in-order DVE stream isn't blocked mid-tail by ihalf1's
                  # wpos muls
                  phase2(0)
                  phase2(1)
                  ps2.__exit__(None, None, None)
                  if stage >= 4:
                      phase3(0)
                      phase3(1)

              if stage >= 4:
                  # loss = sum(mask * (ln(neg) - ln(pos)))
                  lpos = bandp.tile([128, T], F32, tag="lpos")
                  nc.scalar.activation(lpos[:], sc_pos[:], AF.Ln)
                  lneg = bandp.tile([128, T], F32, tag="lneg")
                  nc.scalar.activation(lneg[:], sc_neg[:], AF.Ln)
                  dl = bandp.tile([128, T], F32, tag="dl")
                  nc.vector.tensor_sub(dl[:], lneg[:], lpos[:])
                  mk = bandp.tile([128, T], F32, tag="mk")
                  nc.sync.dma_start(mk[:], i_mask[:])
                  nc.vector.tensor_mul(dl[:], dl[:], mk[:])
                  lsum = bandp.tile([128, 1], F32, tag="lsum")
                  nc.vector.tensor_reduce(lsum[:], dl[:], mybir.AxisListType.X,
                                          ALU.add)
                  nc.sync.dma_start(o_loss[:], lsum[:])
              elif stage == 1:
                  dbg = strm2.tile([128, 1], F32, tag="dbg")
                  nc.vector.tensor_copy(dbg[:], kpT_band[:, 0:1])
                  nc.sync.dma_start(o_loss[:], dbg[:])
              elif stage == 2:
                  tmpld = strm2.tile([128, 1], F8, tag="tmpld")
                  nc.sync.dma_start(tmpld[:],
                                    agout[NW - 1][NCORES - 1][:, 0:1])
                  dbg = strm2.tile([128, 1], F32, tag="dbg")
                  nc.vector.tensor_copy(dbg[:], tmpld[:])
                  nc.sync.dma_start(o_loss[:], dbg[:])
              elif stage == 3:
                  dbg = strm2.tile([128, 1], F32, tag="dbg")
                  nc.vector.tensor_copy(dbg[:], kj_all[:, 0:1])
                  nc.sync.dma_start(o_loss[:], dbg[:])

    nc.compile()
    return nc


_CACHE = {}


def get_compiled(T, QT, bias_nonzero, stage=4, reps=1):
    key = (T, QT, bias_nonzero, stage, reps)
    if key not in _CACHE:
        _CACHE[key] = build_program(T, QT, bias_nonzero, stage, reps)
    return _CACHE[key]


def make_in_maps(per_core):
    return [dict(d) for d in per_core]


def kernel(x, node_to_par, p_weight, A_P, W, b):
    per_core, meta = prep_inputs(x, node_to_par, p_weight, A_P, W, b)
    nc = get_compiled(meta["T"], meta["QT"], meta["bias_nonzero"])
    res = bass_utils.run_bass_kernel_spmd(
        nc, make_in_maps(per_core), core_ids=list(range(NCORES)))
    total = np.float64(0.0)
    for c in range(NCORES):
        total += np.asarray(res.results[c]["loss_part"], np.float64).sum()
    return np.float32(total / meta["N"])
